# revision 11
# baseline (speedup 1.0000x reference)
"""Grouped-expert FFN (MoE) Trainium2 kernel.

Problem: E=64 experts, each x[1024,512] @ w1[512,2048] -> +b1 -> gelu(erf)
-> @ w2[2048,512] -> +b2, rows >= valid_load[e] zeroed.

Strategy:
 - Expert parallelism over 8 cores with COLUMN SPLITTING for load balance:
   a slot s (same chunk widths on every core, SPMD) holds, per core, a
   window of one expert's valid columns. Large experts are split into
   pieces that land on several cores. The piece->slot packing is chosen
   by a simulated anneal whose objective is a forward DMA/PE pipeline
   simulation (weights are 4MB per slot regardless of width, so slots
   must stay wide enough that compute hides the weight stream).
 - x, w1, w2 and the intermediate h are bf16 (halves HBM traffic); PSUM
   accumulation, biases and y stay fp32. End-to-end error ~3e-3 relative
   vs the 2e-2 gate.
 - Host transposes x per expert (xT [D,C]) so the device contracts over D
   with zero on-chip transposes: GEMM1 computes hT = w1-tiles.T @ xT
   (stationary w1 k/m tile, moving xT), GEMM2 computes yT = w2-tiles @ hT.
   Both biases land on the partition axis -> free via ACT activation bias.
 - DMA order is prologue-aware (slot0: w1 m-tile 0 + first x chunk on the
   lightly-used vector ring before the bulk weights) and the y writeback
   is split per output m-tile so the kernel tail is short.
 - Host assembles the full output with zeros for the masked rows.
"""

import numpy as np
import ml_dtypes
import math
import random

import concourse.bass as bass
import concourse.bacc as bacc
import concourse.tile as tile
from concourse import mybir
from concourse.bass_utils import run_bass_kernel_spmd

E, CAP, D, H = 64, 1024, 512, 2048
N_CORES = 8
CHUNK = 512                      # max moving-operand / PSUM-bank width
KTILES1 = D // 128               # 4  (contraction tiles of GEMM1)
MTILES1 = H // 128               # 16 (output partition tiles of GEMM1)
KTILES2 = H // 128               # 16 (contraction tiles of GEMM2)
MTILES2 = D // 128               # 4  (output partition tiles of GEMM2)
GRAN = 8                         # slot caps rounded up to multiple of 8

F32 = mybir.dt.float32
BF16 = mybir.dt.bfloat16
NP_BF16 = ml_dtypes.bfloat16

_PROGRAM_CACHE: dict[tuple, object] = {}
_ASSIGN_CACHE: dict[bytes, tuple] = {}
LAST_RESULT = None               # test harness introspection


def _chunks_of(cap: int) -> tuple:
    """Split a slot width into <=CHUNK near-even pieces."""
    if cap <= 0:
        return ()
    n = -(-cap // CHUNK)
    base, rem = divmod(cap, n)
    return tuple(base + (1 if i < rem else 0) for i in range(n))


# ---------------------------------------------------------------- balance

_BW = 0.24e3                     # input-side DMA bytes/us share (240 GB/s)
_WBYTES = 4 * 2**20              # bf16 w1+w2 bytes per slot
_LOOK = 3                        # weight prefetch lookahead (pool bufs=4)


def _slot_pe(cap):
    return sum(128 * max(W / 2.4 + 3.0, 30.0) for W in _chunks_of(cap))


def _span_sim(caps):
    """Forward-simulate the DMA/PE pipeline for slots in given order."""
    S = len(caps)
    pe_end = [0.0] * S
    cursor = 0.0
    prev = 0.0
    for s in range(S):
        byts = _WBYTES + caps[s] * 1024
        begin = max(cursor, pe_end[s - _LOOK] if s >= _LOOK else 0.0)
        done = begin + byts / _BW
        cursor = done
        start = max(prev, done, 8000.0 if s == 0 else 0.0)
        prev = start + _slot_pe(caps[s])
        pe_end[s] = prev
    return pe_end[-1] + 4000.0


def _caps_from_counts(v, counts):
    pieces = []
    for i, (vi, c) in enumerate(zip(v, counts)):
        if vi <= 0:
            continue
        base, rem = divmod(vi, c)
        for j in range(c):
            pieces.append([base + (1 if j < rem else 0), i])
    pieces.sort(key=lambda p: -p[0])
    while len(pieces) % N_CORES:
        pieces.append([0, -1])
    return pieces


def _shave(pieces):
    """Shrink block maxima by shifting columns to sibling pieces."""
    pieces = [list(p) for p in pieces]
    for _ in range(60):
        nb = len(pieces) // N_CORES
        moved = False
        for b in range(nb):
            blk = pieces[b * N_CORES:(b + 1) * N_CORES]
            mx = max(p[0] for p in blk)
            for p in blk:
                if p[0] == mx and p[1] >= 0:
                    second = max(q[0] for q in blk if q is not p)
                    for ob in range(nb):
                        if ob == b:
                            continue
                        oblk = pieces[ob * N_CORES:(ob + 1) * N_CORES]
                        omx = max(q[0] for q in oblk)
                        for q in oblk:
                            if q[1] == p[1] and q[0] < omx:
                                delta = min(p[0] - second, omx - q[0])
                                if delta > 0:
                                    p[0] -= delta
                                    q[0] += delta
                                    moved = True
                    break
        pieces.sort(key=lambda p: -p[0])
        if not moved:
            break
    return pieces


def _plan(v):
    """Choose slot caps + per-(core,slot) expert pieces for valid_load v."""
    key = v.tobytes()
    if key in _ASSIGN_CACHE:
        return _ASSIGN_CACHE[key]

    def evaluate(counts, maxslots=16):
        if sum(counts) > N_CORES * maxslots:
            return 1e18, None, None
        p = _shave(_caps_from_counts(v, counts))
        caps = [-(-p[b][0] // GRAN) * GRAN
                for b in range(0, len(p), N_CORES) if p[b][0] > 0]
        if not caps:
            return 1e18, None, None
        return _span_sim(caps), caps, p

    best = None
    for seed in range(4):
        rng = random.Random(seed)
        cur = [1] * E
        curC, caps0, p0 = evaluate(cur)
        if best is None:
            best = (curC, caps0, p0, cur[:])
        NIT = 3000
        for it in range(NIT):
            Tmp = 6000.0 * (60.0 / 6000.0) ** (it / NIT)
            c2 = cur[:]
            for _ in range(1 + (rng.random() < 0.35)):
                i = rng.randrange(E)
                if c2[i] == 1 or rng.random() < 0.6:
                    c2[i] += 1
                else:
                    c2[i] -= 1
            co, caps, p = evaluate(c2)
            if caps is None:
                continue
            if co < curC or rng.random() < math.exp(-(co - curC) / Tmp):
                cur, curC = c2, co
                if co < best[0]:
                    best = (co, caps, p, c2[:])
    _, caps, pieces, _ = best

    # assignment: block b piece i -> (core i, slot b); per-expert column
    # ranges handed out cumulatively in piece order
    nslots = len(caps)
    offsets = np.zeros(E, np.int64)
    assign = [[None] * nslots for _ in range(N_CORES)]
    for b in range(nslots):
        blk = pieces[b * N_CORES:(b + 1) * N_CORES]
        for c, (size, e) in enumerate(blk):
            if e < 0 or size <= 0:
                continue
            a = int(offsets[e])
            ln = int(size)
            offsets[e] += ln
            w = min(a, CAP - caps[b])          # compute window start
            assign[c][b] = (int(e), a, ln, w)
    for e in range(E):
        assert offsets[e] == max(int(v[e]), 0), (e, offsets[e], v[e])

    result = (tuple(int(c) for c in caps), assign)
    _ASSIGN_CACHE[key] = result
    return result


# ---------------------------------------------------------------- program

def _build_program(chunk_widths: tuple):
    """One SPMD program; slot s runs chunks of widths chunk_widths[s]."""
    nc = bacc.Bacc(None, target_bir_lowering=False)

    caps = [sum(ws) for ws in chunk_widths]
    offs = np.concatenate([[0], np.cumsum(caps)]).astype(int)
    totcap = int(offs[-1])
    S = len(chunk_widths)

    xt = nc.dram_tensor("xt", [KTILES1, 128, totcap], BF16, kind="ExternalInput")
    w1g = nc.dram_tensor("w1g", [S, MTILES1, 128, KTILES1, 128], BF16,
                         kind="ExternalInput")
    w2g = nc.dram_tensor("w2g", [S, MTILES2, 128, KTILES2, 128], BF16,
                         kind="ExternalInput")
    b1g = nc.dram_tensor("b1g", [S, 128, MTILES1], F32, kind="ExternalInput")
    b2g = nc.dram_tensor("b2g", [S, 128, MTILES2], F32, kind="ExternalInput")
    yt = nc.dram_tensor("yt", [MTILES2, 128, totcap], F32, kind="ExternalOutput")

    Gelu = mybir.ActivationFunctionType.Gelu
    Ident = mybir.ActivationFunctionType.Identity

    with tile.TileContext(nc) as tc:
        with (
            tc.tile_pool(name="w1p", bufs=4) as w1p,
            tc.tile_pool(name="w2p", bufs=4) as w2p,
            tc.tile_pool(name="bp", bufs=2) as bp,
            tc.tile_pool(name="xp", bufs=3) as xp,
            tc.tile_pool(name="hp", bufs=2) as hp,
            tc.tile_pool(name="yp", bufs=2) as yp,
            tc.tile_pool(name="ps_h", bufs=4, space="PSUM") as ps_h,
            tc.tile_pool(name="ps_y", bufs=4, space="PSUM") as ps_y,
        ):
            for s, widths in enumerate(chunk_widths):
                if not widths:
                    continue
                base = int(offs[s])
                b1_t = bp.tile([128, MTILES1], F32, tag="b1")
                b2_t = bp.tile([128, MTILES2], F32, tag="b2")
                w1_t = w1p.tile([128, MTILES1, KTILES1, 128], BF16, tag="w1")
                w2_t = w2p.tile([128, MTILES2, KTILES2, 128], BF16, tag="w2")
                # weight streams alternate between the sync and gpsimd rings
                # so no single ring's descriptor stream becomes the choke;
                # slot0 is split fine (per m-tile) to overlap the first
                # matmuls with the rest of its own w1 stream
                wring = nc.sync if s % 2 == 0 else nc.gpsimd
                w1v = w1g[s].rearrange("m p k j -> p m k j")
                w2v = w2g[s].rearrange("m p k j -> p m k j")
                if s == 0:
                    # DMA bandwidth is shared fairly PER dma_start, so the
                    # gating set (b1, w1 m-tile 0, first x chunk) is split
                    # into many small instructions to grab a large share,
                    # while bulk streams ride as few instructions
                    nc.sync.dma_start(out=b1_t, in_=b1g[s])
                    nc.gpsimd.dma_start(out=b2_t, in_=b2g[s])
                    nc.sync.dma_start(out=w1_t[:, 0, :2], in_=w1g[s, 0][:, :2])
                    nc.gpsimd.dma_start(out=w1_t[:, 0, 2:], in_=w1g[s, 0][:, 2:])
                else:
                    wring.dma_start(out=b1_t, in_=b1g[s])
                    wring.dma_start(out=b2_t, in_=b2g[s])
                    wring.dma_start(out=w1_t[:, 0], in_=w1g[s, 0])

                x_ts = []
                col = base
                for j, W in enumerate(widths):
                    x_t = xp.tile([128, KTILES1, CHUNK], BF16, tag="x")
                    if s == 0 and j == 0:
                        hw = W // 2
                        for k in range(KTILES1):
                            ring = nc.gpsimd if k % 2 else nc.sync
                            ring.dma_start(
                                out=x_t[:, k, :hw], in_=xt[k][:, col:col + hw])
                            ring.dma_start(
                                out=x_t[:, k, hw:W],
                                in_=xt[k][:, col + hw:col + W])
                    else:
                        nc.sync.dma_start(
                            out=x_t[:, :, :W],
                            in_=xt.rearrange("k p c -> p k c")[:, :, col:col + W],
                        )
                    x_ts.append((x_t, W, col))
                    col += W
                    if j == 0:
                        if s == 0:
                            # stream w1 in 2-m-tile pieces right behind the
                            # m-loop; w2 is emitted after the first GELU so
                            # its 2MB stays out of the gating window
                            for m in range(1, MTILES1, 2):
                                hi = min(m + 2, MTILES1)
                                nc.sync.dma_start(
                                    out=w1_t[:, m:hi], in_=w1v[:, m:hi])
                        else:
                            half = MTILES1 // 2
                            wring.dma_start(
                                out=w1_t[:, 1:half], in_=w1v[:, 1:half])
                            wring.dma_start(
                                out=w1_t[:, half:], in_=w1v[:, half:])
                            for dm in range(MTILES2):
                                nc.scalar.dma_start(
                                    out=w2_t[:, dm], in_=w2v[:, dm])

                for ci, (x_t, W, col) in enumerate(x_ts):
                    h_t = hp.tile([128, KTILES2, CHUNK], BF16, tag="h")
                    for m in range(MTILES1):
                        ps = ps_h.tile([128, CHUNK], F32, tag="psh")
                        for k in range(KTILES1):
                            nc.tensor.matmul(
                                ps[:, :W],
                                lhsT=w1_t[:, m, k],
                                rhs=x_t[:, k, :W],
                                start=(k == 0),
                                stop=(k == KTILES1 - 1),
                            )
                        nc.scalar.activation(
                            h_t[:, m, :W], ps[:, :W], Gelu, bias=b1_t[:, m:m + 1]
                        )
                        if s == 0 and ci == 0 and m == 0:
                            for dm in range(MTILES2):
                                nc.scalar.dma_start(
                                    out=w2_t[:, dm], in_=w2v[:, dm])

                    y_t = yp.tile([128, MTILES2, CHUNK], F32, tag="y")
                    for dm in range(MTILES2):
                        ps2 = ps_y.tile([128, CHUNK], F32, tag="psy")
                        for k in range(KTILES2):
                            nc.tensor.matmul(
                                ps2[:, :W],
                                lhsT=w2_t[:, dm, k],
                                rhs=h_t[:, k, :W],
                                start=(k == 0),
                                stop=(k == KTILES2 - 1),
                            )
                        nc.scalar.activation(
                            y_t[:, dm, :W], ps2[:, :W], Ident, bias=b2_t[:, dm:dm + 1]
                        )
                    nc.gpsimd.dma_start(
                        out=yt.rearrange("m p c -> p m c")[:, :, col:col + W],
                        in_=y_t[:, :, :W],
                    )

    nc.compile()
    return nc


def kernel(packed_inputs, valid_load, w1, b1, w2, b2, _trace=False, **_):
    global LAST_RESULT
    packed_inputs = np.asarray(packed_inputs, np.float32)
    w1 = np.asarray(w1, np.float32)
    b1 = np.asarray(b1, np.float32)
    w2 = np.asarray(w2, np.float32)
    b2 = np.asarray(b2, np.float32)
    v = np.asarray(valid_load).astype(np.int64)
    v = np.clip(v, 0, CAP)

    out = np.zeros((E, CAP, D), np.float32)
    if int(v.max()) <= 0:
        return out

    caps, assign = _plan(v)
    chunk_widths = tuple(_chunks_of(c) for c in caps)
    nslots = len(caps)
    offs = np.concatenate([[0], np.cumsum(caps)]).astype(int)
    totcap = int(offs[-1])

    key = chunk_widths
    if key not in _PROGRAM_CACHE:
        _PROGRAM_CACHE[key] = _build_program(chunk_widths)
    nc = _PROGRAM_CACHE[key]

    xb = packed_inputs.astype(NP_BF16)
    w1b = w1.astype(NP_BF16)
    w2b = w2.astype(NP_BF16)
    # pre-shaped per-expert views
    xT = np.ascontiguousarray(xb.transpose(0, 2, 1)).reshape(E, KTILES1, 128, CAP)
    w1m = np.ascontiguousarray(
        w1b.reshape(E, KTILES1, 128, MTILES1, 128).transpose(0, 3, 2, 1, 4))
    w2m = np.ascontiguousarray(
        w2b.reshape(E, KTILES2, 128, MTILES2, 128).transpose(0, 3, 2, 1, 4))
    b1m = np.ascontiguousarray(b1.reshape(E, MTILES1, 128).transpose(0, 2, 1))
    b2m = np.ascontiguousarray(b2.reshape(E, MTILES2, 128).transpose(0, 2, 1))

    in_maps = []
    for c in range(N_CORES):
        xt_c = np.zeros((KTILES1, 128, totcap), NP_BF16)
        w1_c = np.zeros((nslots, MTILES1, 128, KTILES1, 128), NP_BF16)
        w2_c = np.zeros((nslots, MTILES2, 128, KTILES2, 128), NP_BF16)
        b1_c = np.zeros((nslots, 128, MTILES1), np.float32)
        b2_c = np.zeros((nslots, 128, MTILES2), np.float32)
        for s in range(nslots):
            item = assign[c][s]
            if item is None:
                continue
            e, a, ln, w = item
            o = int(offs[s])
            xt_c[:, :, o:o + caps[s]] = xT[e][:, :, w:w + caps[s]]
            w1_c[s] = w1m[e]
            w2_c[s] = w2m[e]
            b1_c[s] = b1m[e]
            b2_c[s] = b2m[e]
        in_maps.append({
            "xt": xt_c, "w1g": w1_c, "w2g": w2_c, "b1g": b1_c, "b2g": b2_c,
        })

    res = run_bass_kernel_spmd(nc, in_maps, list(range(N_CORES)), trace=_trace)
    LAST_RESULT = res

    for c in range(N_CORES):
        ytc = res.results[c]["yt"]          # [MTILES2, 128, totcap]
        yflat = ytc.reshape(D, totcap)      # d = 128m+p
        for s in range(nslots):
            item = assign[c][s]
            if item is None:
                continue
            e, a, ln, w = item
            o = int(offs[s]) + (a - w)
            out[e, a:a + ln, :] = yflat[:, o:o + ln].T
    return out


# revision 15
# speedup vs baseline: 1.0052x; 1.0052x over previous
"""Grouped-expert FFN (MoE) Trainium2 kernel.

Problem: E=64 experts, each x[1024,512] @ w1[512,2048] -> +b1 -> gelu(erf)
-> @ w2[2048,512] -> +b2, rows >= valid_load[e] zeroed.

Strategy:
 - Expert parallelism over 8 cores with COLUMN SPLITTING for load balance:
   a slot s (same chunk widths on every core, SPMD) holds, per core, a
   window of one expert's valid columns. Large experts are split into
   pieces that land on several cores. The piece->slot packing is chosen
   by a simulated anneal whose objective is a forward DMA/PE pipeline
   simulation (weights are 4MB per slot regardless of width, so slots
   must stay wide enough that compute hides the weight stream).
 - x, w1, w2 and the intermediate h are bf16 (halves HBM traffic); PSUM
   accumulation, biases and y stay fp32. End-to-end error ~3e-3 relative
   vs the 2e-2 gate.
 - Host transposes x per expert (xT [D,C]) so the device contracts over D
   with zero on-chip transposes: GEMM1 computes hT = w1-tiles.T @ xT
   (stationary w1 k/m tile, moving xT), GEMM2 computes yT = w2-tiles @ hT.
   Both biases land on the partition axis -> free via ACT activation bias.
 - DMA order is prologue-aware (slot0: w1 m-tile 0 + first x chunk on the
   lightly-used vector ring before the bulk weights) and the y writeback
   is split per output m-tile so the kernel tail is short.
 - Host assembles the full output with zeros for the masked rows.
"""

import numpy as np
import ml_dtypes
import math
import random

import concourse.bass as bass
import concourse.bacc as bacc
import concourse.tile as tile
from concourse import mybir
from concourse.bass_utils import run_bass_kernel_spmd

E, CAP, D, H = 64, 1024, 512, 2048
N_CORES = 8
CHUNK = 512                      # max moving-operand / PSUM-bank width
KTILES1 = D // 128               # 4  (contraction tiles of GEMM1)
MTILES1 = H // 128               # 16 (output partition tiles of GEMM1)
KTILES2 = H // 128               # 16 (contraction tiles of GEMM2)
MTILES2 = D // 128               # 4  (output partition tiles of GEMM2)
GRAN = 8                         # slot caps rounded up to multiple of 8

F32 = mybir.dt.float32
BF16 = mybir.dt.bfloat16
NP_BF16 = ml_dtypes.bfloat16

_PROGRAM_CACHE: dict[tuple, object] = {}
_ASSIGN_CACHE: dict[bytes, tuple] = {}
LAST_RESULT = None               # test harness introspection


def _chunks_of(cap: int) -> tuple:
    """Split a slot width into <=CHUNK near-even pieces."""
    if cap <= 0:
        return ()
    n = -(-cap // CHUNK)
    base, rem = divmod(cap, n)
    return tuple(base + (1 if i < rem else 0) for i in range(n))


# ---------------------------------------------------------------- balance

_BW = 0.24e3                     # input-side DMA bytes/us share (240 GB/s)
_WBYTES = 4 * 2**20              # bf16 w1+w2 bytes per slot
_LOOK = 3                        # weight prefetch lookahead (pool bufs=4)


def _slot_pe(cap):
    return sum(128 * max(W / 2.4 + 3.0, 30.0) for W in _chunks_of(cap))


def _span_sim(caps):
    """Forward-simulate the DMA/PE pipeline for slots in given order."""
    S = len(caps)
    pe_end = [0.0] * S
    cursor = 0.0
    prev = 0.0
    for s in range(S):
        byts = _WBYTES + caps[s] * 1024
        begin = max(cursor, pe_end[s - _LOOK] if s >= _LOOK else 0.0)
        done = begin + byts / _BW
        cursor = done
        start = max(prev, done, 8000.0 if s == 0 else 0.0)
        prev = start + _slot_pe(caps[s])
        pe_end[s] = prev
    return pe_end[-1] + 4000.0


def _caps_from_counts(v, counts):
    pieces = []
    for i, (vi, c) in enumerate(zip(v, counts)):
        if vi <= 0:
            continue
        base, rem = divmod(vi, c)
        for j in range(c):
            pieces.append([base + (1 if j < rem else 0), i])
    pieces.sort(key=lambda p: -p[0])
    while len(pieces) % N_CORES:
        pieces.append([0, -1])
    return pieces


def _shave(pieces):
    """Shrink block maxima by shifting columns to sibling pieces."""
    pieces = [list(p) for p in pieces]
    for _ in range(60):
        nb = len(pieces) // N_CORES
        moved = False
        for b in range(nb):
            blk = pieces[b * N_CORES:(b + 1) * N_CORES]
            mx = max(p[0] for p in blk)
            for p in blk:
                if p[0] == mx and p[1] >= 0:
                    second = max(q[0] for q in blk if q is not p)
                    for ob in range(nb):
                        if ob == b:
                            continue
                        oblk = pieces[ob * N_CORES:(ob + 1) * N_CORES]
                        omx = max(q[0] for q in oblk)
                        for q in oblk:
                            if q[1] == p[1] and q[0] < omx:
                                delta = min(p[0] - second, omx - q[0])
                                if delta > 0:
                                    p[0] -= delta
                                    q[0] += delta
                                    moved = True
                    break
        pieces.sort(key=lambda p: -p[0])
        if not moved:
            break
    return pieces


def _plan(v):
    """Choose slot caps + per-(core,slot) expert pieces for valid_load v."""
    key = v.tobytes()
    if key in _ASSIGN_CACHE:
        return _ASSIGN_CACHE[key]

    def evaluate(counts, maxslots=16):
        if sum(counts) > N_CORES * maxslots:
            return 1e18, None, None
        p = _shave(_caps_from_counts(v, counts))
        caps = [-(-p[b][0] // GRAN) * GRAN
                for b in range(0, len(p), N_CORES) if p[b][0] > 0]
        if not caps:
            return 1e18, None, None
        return _span_sim(caps), caps, p

    best = None
    for seed in range(4):
        rng = random.Random(seed)
        cur = [1] * E
        curC, caps0, p0 = evaluate(cur)
        if best is None:
            best = (curC, caps0, p0, cur[:])
        NIT = 3000
        for it in range(NIT):
            Tmp = 6000.0 * (60.0 / 6000.0) ** (it / NIT)
            c2 = cur[:]
            for _ in range(1 + (rng.random() < 0.35)):
                i = rng.randrange(E)
                if c2[i] == 1 or rng.random() < 0.6:
                    c2[i] += 1
                else:
                    c2[i] -= 1
            co, caps, p = evaluate(c2)
            if caps is None:
                continue
            if co < curC or rng.random() < math.exp(-(co - curC) / Tmp):
                cur, curC = c2, co
                if co < best[0]:
                    best = (co, caps, p, c2[:])
    _, caps, pieces, _ = best

    # assignment: block b piece i -> (core i, slot b); per-expert column
    # ranges handed out cumulatively in piece order
    nslots = len(caps)
    offsets = np.zeros(E, np.int64)
    assign = [[None] * nslots for _ in range(N_CORES)]
    for b in range(nslots):
        blk = pieces[b * N_CORES:(b + 1) * N_CORES]
        for c, (size, e) in enumerate(blk):
            if e < 0 or size <= 0:
                continue
            a = int(offsets[e])
            ln = int(size)
            offsets[e] += ln
            w = min(a, CAP - caps[b])          # compute window start
            assign[c][b] = (int(e), a, ln, w)
    for e in range(E):
        assert offsets[e] == max(int(v[e]), 0), (e, offsets[e], v[e])

    result = (tuple(int(c) for c in caps), assign)
    _ASSIGN_CACHE[key] = result
    return result


# ---------------------------------------------------------------- program

def _build_program(chunk_widths: tuple):
    """One SPMD program; slot s runs chunks of widths chunk_widths[s]."""
    nc = bacc.Bacc(None, target_bir_lowering=False)

    caps = [sum(ws) for ws in chunk_widths]
    offs = np.concatenate([[0], np.cumsum(caps)]).astype(int)
    totcap = int(offs[-1])
    S = len(chunk_widths)

    xt = nc.dram_tensor("xt", [KTILES1, 128, totcap], BF16, kind="ExternalInput")
    # weights are p-major so DMA descriptor rows are 2-16KB contiguous
    w1g = nc.dram_tensor("w1g", [S, 128, MTILES1, KTILES1, 128], BF16,
                         kind="ExternalInput")
    w2g = nc.dram_tensor("w2g", [S, 128, MTILES2, KTILES2, 128], BF16,
                         kind="ExternalInput")
    b1g = nc.dram_tensor("b1g", [S, 128, MTILES1], F32, kind="ExternalInput")
    b2g = nc.dram_tensor("b2g", [S, 128, MTILES2], F32, kind="ExternalInput")
    yt = nc.dram_tensor("yt", [MTILES2, 128, totcap], F32, kind="ExternalOutput")

    Gelu = mybir.ActivationFunctionType.Gelu
    Ident = mybir.ActivationFunctionType.Identity

    with tile.TileContext(nc) as tc:
        with (
            tc.tile_pool(name="w1p", bufs=4) as w1p,
            tc.tile_pool(name="w2p", bufs=4) as w2p,
            tc.tile_pool(name="bp", bufs=2) as bp,
            tc.tile_pool(name="xp", bufs=3) as xp,
            tc.tile_pool(name="hp", bufs=2) as hp,
            tc.tile_pool(name="yp", bufs=2) as yp,
            tc.tile_pool(name="ps_h", bufs=4, space="PSUM") as ps_h,
            tc.tile_pool(name="ps_y", bufs=4, space="PSUM") as ps_y,
        ):
            for s, widths in enumerate(chunk_widths):
                if not widths:
                    continue
                base = int(offs[s])
                b1_t = bp.tile([128, MTILES1], F32, tag="b1")
                b2_t = bp.tile([128, MTILES2], F32, tag="b2")
                w1_t = w1p.tile([128, MTILES1, KTILES1, 128], BF16, tag="w1")
                w2_t = w2p.tile([128, MTILES2, KTILES2, 128], BF16, tag="w2")
                # weight streams alternate between the sync and gpsimd rings
                # so no single ring's descriptor stream becomes the choke;
                # slot0 is split fine (per m-tile) to overlap the first
                # matmuls with the rest of its own w1 stream
                wring = nc.sync if s % 2 == 0 else nc.gpsimd
                w1v = w1g[s]
                w2v = w2g[s]
                if s == 0:
                    # DMA bandwidth is shared fairly PER dma_start, so the
                    # gating set (b1, w1 m-tile 0, first x chunk) is split
                    # into many small instructions to grab a large share,
                    # while bulk streams ride as few instructions
                    nc.sync.dma_start(out=b1_t, in_=b1g[s])
                    nc.gpsimd.dma_start(out=b2_t, in_=b2g[s])
                    nc.sync.dma_start(out=w1_t[:, 0, :2], in_=w1v[:, 0, :2])
                    nc.gpsimd.dma_start(out=w1_t[:, 0, 2:], in_=w1v[:, 0, 2:])
                else:
                    wring.dma_start(out=b1_t, in_=b1g[s])
                    wring.dma_start(out=b2_t, in_=b2g[s])
                    wring.dma_start(out=w1_t[:, 0], in_=w1v[:, 0])

                x_ts = []
                col = base
                for j, W in enumerate(widths):
                    x_t = xp.tile([128, KTILES1, CHUNK], BF16, tag="x")
                    if s == 0 and j == 0:
                        hw = W // 2
                        for k in range(KTILES1):
                            ring = nc.gpsimd if k % 2 else nc.sync
                            ring.dma_start(
                                out=x_t[:, k, :hw], in_=xt[k][:, col:col + hw])
                            ring.dma_start(
                                out=x_t[:, k, hw:W],
                                in_=xt[k][:, col + hw:col + W])
                    else:
                        nc.sync.dma_start(
                            out=x_t[:, :, :W],
                            in_=xt.rearrange("k p c -> p k c")[:, :, col:col + W],
                        )
                    x_ts.append((x_t, W, col))
                    col += W
                    if j == 0:
                        if s == 0:
                            # stream w1 in 2-m-tile pieces right behind the
                            # m-loop; w2 is emitted after the first GELU so
                            # its 2MB stays out of the gating window
                            for m in range(1, MTILES1, 2):
                                hi = min(m + 2, MTILES1)
                                nc.sync.dma_start(
                                    out=w1_t[:, m:hi], in_=w1v[:, m:hi])
                        else:
                            half = MTILES1 // 2
                            wring.dma_start(
                                out=w1_t[:, 1:half], in_=w1v[:, 1:half])
                            wring.dma_start(
                                out=w1_t[:, half:], in_=w1v[:, half:])
                            for dm in range(MTILES2):
                                nc.scalar.dma_start(
                                    out=w2_t[:, dm], in_=w2v[:, dm])

                for ci, (x_t, W, col) in enumerate(x_ts):
                    h_t = hp.tile([128, KTILES2, CHUNK], BF16, tag="h")
                    for m in range(MTILES1):
                        ps = ps_h.tile([128, CHUNK], F32, tag="psh")
                        for k in range(KTILES1):
                            nc.tensor.matmul(
                                ps[:, :W],
                                lhsT=w1_t[:, m, k],
                                rhs=x_t[:, k, :W],
                                start=(k == 0),
                                stop=(k == KTILES1 - 1),
                            )
                        nc.scalar.activation(
                            h_t[:, m, :W], ps[:, :W], Gelu, bias=b1_t[:, m:m + 1]
                        )
                        if s == 0 and ci == 0 and m == 0:
                            for dm in range(MTILES2):
                                nc.scalar.dma_start(
                                    out=w2_t[:, dm], in_=w2v[:, dm])

                    y_t = yp.tile([128, MTILES2, CHUNK], F32, tag="y")
                    for dm in range(MTILES2):
                        ps2 = ps_y.tile([128, CHUNK], F32, tag="psy")
                        for k in range(KTILES2):
                            nc.tensor.matmul(
                                ps2[:, :W],
                                lhsT=w2_t[:, dm, k],
                                rhs=h_t[:, k, :W],
                                start=(k == 0),
                                stop=(k == KTILES2 - 1),
                            )
                        nc.scalar.activation(
                            y_t[:, dm, :W], ps2[:, :W], Ident, bias=b2_t[:, dm:dm + 1]
                        )
                    nc.gpsimd.dma_start(
                        out=yt.rearrange("m p c -> p m c")[:, :, col:col + W],
                        in_=y_t[:, :, :W],
                    )

    nc.compile()
    return nc


def kernel(packed_inputs, valid_load, w1, b1, w2, b2, _trace=False, **_):
    global LAST_RESULT
    packed_inputs = np.asarray(packed_inputs, np.float32)
    w1 = np.asarray(w1, np.float32)
    b1 = np.asarray(b1, np.float32)
    w2 = np.asarray(w2, np.float32)
    b2 = np.asarray(b2, np.float32)
    v = np.asarray(valid_load).astype(np.int64)
    v = np.clip(v, 0, CAP)

    out = np.zeros((E, CAP, D), np.float32)
    if int(v.max()) <= 0:
        return out

    caps, assign = _plan(v)
    chunk_widths = tuple(_chunks_of(c) for c in caps)
    nslots = len(caps)
    offs = np.concatenate([[0], np.cumsum(caps)]).astype(int)
    totcap = int(offs[-1])

    key = chunk_widths
    if key not in _PROGRAM_CACHE:
        _PROGRAM_CACHE[key] = _build_program(chunk_widths)
    nc = _PROGRAM_CACHE[key]

    xb = packed_inputs.astype(NP_BF16)
    w1b = w1.astype(NP_BF16)
    w2b = w2.astype(NP_BF16)
    # pre-shaped per-expert views (weights p-major: [p, m, k, j])
    xT = np.ascontiguousarray(xb.transpose(0, 2, 1)).reshape(E, KTILES1, 128, CAP)
    w1m = np.ascontiguousarray(
        w1b.reshape(E, KTILES1, 128, MTILES1, 128).transpose(0, 2, 3, 1, 4))
    w2m = np.ascontiguousarray(
        w2b.reshape(E, KTILES2, 128, MTILES2, 128).transpose(0, 2, 3, 1, 4))
    b1m = np.ascontiguousarray(b1.reshape(E, MTILES1, 128).transpose(0, 2, 1))
    b2m = np.ascontiguousarray(b2.reshape(E, MTILES2, 128).transpose(0, 2, 1))

    in_maps = []
    for c in range(N_CORES):
        xt_c = np.zeros((KTILES1, 128, totcap), NP_BF16)
        w1_c = np.zeros((nslots, 128, MTILES1, KTILES1, 128), NP_BF16)
        w2_c = np.zeros((nslots, 128, MTILES2, KTILES2, 128), NP_BF16)
        b1_c = np.zeros((nslots, 128, MTILES1), np.float32)
        b2_c = np.zeros((nslots, 128, MTILES2), np.float32)
        for s in range(nslots):
            item = assign[c][s]
            if item is None:
                continue
            e, a, ln, w = item
            o = int(offs[s])
            xt_c[:, :, o:o + caps[s]] = xT[e][:, :, w:w + caps[s]]
            w1_c[s] = w1m[e]
            w2_c[s] = w2m[e]
            b1_c[s] = b1m[e]
            b2_c[s] = b2m[e]
        in_maps.append({
            "xt": xt_c, "w1g": w1_c, "w2g": w2_c, "b1g": b1_c, "b2g": b2_c,
        })

    res = run_bass_kernel_spmd(nc, in_maps, list(range(N_CORES)), trace=_trace)
    LAST_RESULT = res

    for c in range(N_CORES):
        ytc = res.results[c]["yt"]          # [MTILES2, 128, totcap]
        yflat = ytc.reshape(D, totcap)      # d = 128m+p
        for s in range(nslots):
            item = assign[c][s]
            if item is None:
                continue
            e, a, ln, w = item
            o = int(offs[s]) + (a - w)
            out[e, a:a + ln, :] = yflat[:, o:o + ln].T
    return out
